# revision 1
# baseline (speedup 1.0000x reference)
"""RWKV v4 block (nn_Block_15109694947416) on 8 TRN2 NeuronCores.

Strategy:
- Data-parallel over B: core i processes batch i (B=8). No collectives.
- Activations live channel-major [C, T] on-chip: matmuls contract over the
  partition dim natively (lhsT = weight in its DRAM layout), the token-shift
  is a free-dim offset, and the WKV recurrence maps onto the hardware
  tensor_tensor_scan (state = ew*state + data) along the free dim.
- Host pre-transposes x[b] -> [C, T] and precomputes ew=exp(-exp(time_decay)),
  eu=exp(time_first). WKV is computed unstabilized in fp32 which is exact
  (validated ~1e-6 vs the stabilized reference): k is small, w<0.
- time-mix matmuls in float32r (1 cyc/row at N>=256, ~1.4e-4 rel err),
  FFN matmuls in bf16 (~2e-3 on a small additive branch).
- T processed in 8 chunks of 256 columns; scan/token-shift state carried
  across chunks via [128, 8, 1] carry tiles.
"""

import numpy as np
import ml_dtypes

B, T, C = 8, 2048, 1024
TC = 256                 # time chunk
NCH = T // TC            # chunks
CB = C // 128            # channel blocks (8)
FB = 4 * C // 128        # ffn hidden blocks (32)
EPS = 1e-5

_CACHE = {}


def _bcast_free(ap, n):
    """[128,1] AP -> [128,n] stride-0 broadcast along free dim."""
    import concourse.bass as bass
    return bass.AP(tensor=ap.tensor, offset=ap.offset, ap=[ap.ap[0], [0, n]])


def _bcast_mid(ap, nmid):
    """[128,N] AP -> [128,nmid,N] stride-0 broadcast of a middle dim."""
    import concourse.bass as bass
    return bass.AP(tensor=ap.tensor, offset=ap.offset,
                   ap=[ap.ap[0], [0, nmid], ap.ap[1]])


def _build():
    import concourse.bass as bass
    import concourse.bacc as bacc
    import concourse.tile as tile
    from concourse import mybir

    f32 = mybir.dt.float32
    f32r = mybir.dt.float32r
    bf16 = mybir.dt.bfloat16
    AF = mybir.ActivationFunctionType
    OP = mybir.AluOpType

    nc = bacc.Bacc(None, target_bir_lowering=False, debug=False)

    xT = nc.dram_tensor("xT", [C, T], f32r, kind="ExternalInput")
    cvecs = nc.dram_tensor("cvecs", [128, CB, 12], f32, kind="ExternalInput")
    ones_in = nc.dram_tensor("ones128", [128], f32r, kind="ExternalInput")
    Wk = nc.dram_tensor("Wk", [C, C], f32r, kind="ExternalInput")
    Wv = nc.dram_tensor("Wv", [C, C], f32r, kind="ExternalInput")
    Wr = nc.dram_tensor("Wr", [C, C], f32r, kind="ExternalInput")
    Wo = nc.dram_tensor("Wo", [C, C], bf16, kind="ExternalInput")
    fWk = nc.dram_tensor("fWk", [C, 4 * C], bf16, kind="ExternalInput")
    fWv = nc.dram_tensor("fWv", [4 * C, C], bf16, kind="ExternalInput")
    fWr = nc.dram_tensor("fWr", [C, C], bf16, kind="ExternalInput")
    outT = nc.dram_tensor("outT", [C, T], f32, kind="ExternalOutput")


    def dma8(out_t, in_ap, parts=8):
        """Split a [128, M, N] transfer along the middle dim across DMA queues."""
        M = out_t.shape[1]
        step = max(1, M // parts)
        for i in range(0, M, step):
            j = min(i + step, M)
            nc.sync.dma_start(out=out_t[:, i:j, :], in_=in_ap[:, i:j, :])


    def dma8_out(dram_ap, sb_t, parts=8):
        M = sb_t.shape[1]
        step = max(1, M // parts)
        for i in range(0, M, step):
            j = min(i + step, M)
            nc.sync.dma_start(out=dram_ap[:, i:j, :], in_=sb_t[:, i:j, :])

    # cvec row indices
    LN1G, LN1B, LN2G, LN2B, TMK, TMV, TMR, FTMK, FTMR, EW, EU, _ = range(12)

    with tile.TileContext(nc) as tc:
        import contextlib
        with contextlib.ExitStack() as ctx:
            consts = ctx.enter_context(tc.tile_pool(name="consts", bufs=1))
            dramp = ctx.enter_context(tc.tile_pool(name="dram", bufs=1, space="DRAM"))

            cv = consts.tile([128, CB, 12], f32)
            nc.sync.dma_start(out=cv, in_=cvecs[:, :, :])
            ones_k = consts.tile([128, 1], f32r)    # lhsT for column sums
            nc.sync.dma_start(out=ones_k, in_=ones_in.rearrange("(p o) -> p o", o=1))
            ones_b = consts.tile([1, 128], f32r)    # lhsT for row broadcast
            nc.sync.dma_start(out=ones_b, in_=ones_in.rearrange("(o p) -> o p", o=1))

            eps_t = consts.tile([1, 1], f32)
            nc.vector.memset(eps_t, EPS)
            carryH = consts.tile([128, CB, 1], f32)
            carryG = consts.tile([128, CB, 1], f32)
            carryA = consts.tile([128, CB, 1], f32)
            carryB = consts.tile([128, CB, 1], f32)
            for c in (carryH, carryG, carryA, carryB):
                nc.vector.memset(c, 0.0)

            x2d = dramp.tile([NCH, 128, CB, TC], f32)

            def layernorm(pools, x_t, g_row, b_row, h_t):
                """x_t: [128, CB, TC] f32r tile -> h_t[:, :, 1:TC+1] fp32.

                Per-token stats via PE ones-matmuls (cross-partition sums),
                broadcast back via K=1 matmuls.
                """
                sbuf, ps_stat, ps_bc, scratch, sq_tag, s1_tag = pools
                sq = sbuf.tile([128, CB, TC], f32r, tag=sq_tag)
                nc.scalar.activation(out=sq, in_=x_t.bitcast(f32), func=AF.Square)
                st = ps_stat.tile([1, 2 * TC], f32)
                for cb in range(CB):
                    nc.tensor.matmul(st[:, 0:TC], ones_k, x_t[:, cb, :],
                                     start=(cb == 0), stop=(cb == CB - 1))
                for cb in range(CB):
                    nc.tensor.matmul(st[:, TC:2 * TC], ones_k, sq[:, cb, :],
                                     start=(cb == 0), stop=(cb == CB - 1))
                rows = scratch.tile([1, 2 * TC], f32r, tag="rows")
                tmp = scratch.tile([1, 2 * TC], f32, tag="rtmp")
                rowf = rows.bitcast(f32)
                # m = sum/C  (f32r-typed out: consumed by broadcast matmul)
                nc.vector.tensor_scalar_mul(rows[:, 0:TC], st[:, 0:TC], 1.0 / C)
                # m^2
                nc.vector.tensor_mul(tmp[:, 0:TC], rowf[:, 0:TC], rowf[:, 0:TC])
                # var = sumsq/C - m^2
                nc.vector.scalar_tensor_tensor(
                    out=tmp[:, TC:2 * TC], in0=st[:, TC:2 * TC], scalar=1.0 / C,
                    in1=tmp[:, 0:TC], op0=OP.mult, op1=OP.subtract)
                # rstd = 1/sqrt(var + eps)
                nc.scalar.activation(out=tmp[:, TC:2 * TC], in_=tmp[:, TC:2 * TC],
                                     func=AF.Sqrt, bias=eps_t[:, :])
                nc.vector.reciprocal_approx_fast(out=tmp[:, 0:TC],
                                                 in_=tmp[:, TC:2 * TC])
                nc.vector.tensor_copy(out=rows[:, TC:2 * TC], in_=tmp[:, 0:TC])
                # broadcast m and rstd across partitions
                mb = ps_bc.tile([128, TC], f32, tag="mb")
                nc.tensor.matmul(mb, ones_b, rows[:, 0:TC])
                rb = ps_bc.tile([128, TC], f32, tag="rb")
                nc.tensor.matmul(rb, ones_b, rows[:, TC:2 * TC])
                s1 = sbuf.tile([128, CB, TC], f32, tag=s1_tag)
                nc.vector.tensor_sub(s1, x_t.bitcast(f32), _bcast_mid(mb, CB))
                nc.vector.tensor_mul(s1, s1, _bcast_mid(rb, CB))
                for cb in range(CB):
                    nc.scalar.activation(
                        out=h_t[:, cb, 1:TC + 1], in_=s1[:, cb, :],
                        func=AF.Identity, bias=b_row(cb), scale=g_row(cb))

            # ================= Phase 1a: time-mix k/v/r =================
            import os
            _PH = os.environ.get("KPHASES", "12")
            ekd = dramp.tile([NCH, 128, CB, TC], f32, tag="ekd")
            ekvd = dramp.tile([NCH, 128, CB, TC], f32, tag="ekvd")
            rsd1 = dramp.tile([NCH, 128, CB, TC], bf16, tag="rsd1")
            if "1" in _PH:
              with contextlib.ExitStack() as p1:
                wpool = p1.enter_context(tc.tile_pool(name="w1", bufs=1))
                act = p1.enter_context(tc.tile_pool(name="act1", bufs=1))
                dbl = p1.enter_context(tc.tile_pool(name="dbl1", bufs=2))
                scratch = p1.enter_context(tc.tile_pool(name="scr1", bufs=1))
                ps_ev = p1.enter_context(tc.tile_pool(name="ps_ev", bufs=4, space="PSUM"))
                ps_stat = p1.enter_context(tc.tile_pool(name="ps_st", bufs=1, space="PSUM"))
                ps_bc = p1.enter_context(tc.tile_pool(name="ps_bc", bufs=1, space="PSUM"))

                wk_t = wpool.tile([128, CB, C], f32r, tag="wk")
                dma8(wk_t, Wk.rearrange("(a p) m -> p a m", p=128))
                wv_t = wpool.tile([128, CB, C], f32r, tag="wv")
                dma8(wv_t, Wv.rearrange("(a p) m -> p a m", p=128))
                wr_t = wpool.tile([128, CB, C], f32r, tag="wr")
                dma8(wr_t, Wr.rearrange("(a p) m -> p a m", p=128))

                for ic in range(NCH):
                    t0 = ic * TC
                    x_t = dbl.tile([128, CB, TC], f32r, tag="x")
                    dma8(x_t, xT.rearrange("(cb p) t -> p cb t", p=128)[:, :, t0:t0 + TC], parts=4)
                    h_t = dbl.tile([128, CB, TC + 1], f32, tag="h")
                    nc.vector.tensor_copy(out=h_t[:, :, 0:1], in_=carryH)
                    layernorm((dbl, ps_stat, ps_bc, scratch, "d", "d"), x_t,
                              lambda cb: cv[:, cb, LN1G:LN1G + 1],
                              lambda cb: cv[:, cb, LN1B:LN1B + 1], h_t)
                    nc.vector.tensor_copy(out=carryH, in_=h_t[:, :, TC:TC + 1])

                    d_t = dbl.tile([128, CB, TC], f32, tag="d")
                    nc.vector.tensor_sub(d_t, h_t[:, :, 1:TC + 1], h_t[:, :, 0:TC])

                    ek = dbl.tile([128, CB, TC], f32, tag="ek")
                    ekv = dbl.tile([128, CB, TC], f32, tag="ekv")
                    rsig = dbl.tile([128, CB, TC], bf16, tag="rsig")

                    for which, w_t, tmrow in (("k", wk_t, TMK), ("v", wv_t, TMV),
                                              ("r", wr_t, TMR)):
                        in_t = dbl.tile([128, CB, TC], f32r, tag="min")
                        for cb in range(CB):
                            nc.vector.scalar_tensor_tensor(
                                out=in_t[:, cb, :], in0=d_t[:, cb, :],
                                scalar=cv[:, cb, tmrow:tmrow + 1],
                                in1=h_t[:, cb, 0:TC], op0=OP.mult, op1=OP.add)
                        for co in range(CB):
                            ps = ps_ev.tile([128, TC], f32, tag="ev")
                            csl = slice(co * 128, (co + 1) * 128)
                            for a in range(CB):
                                nc.tensor.matmul(ps, w_t[:, a, csl], in_t[:, a, :],
                                                 start=(a == 0), stop=(a == CB - 1))
                            if which == "k":
                                nc.scalar.activation(out=ek[:, co, :], in_=ps, func=AF.Exp)
                            elif which == "v":
                                nc.vector.tensor_mul(ekv[:, co, :], ek[:, co, :], ps)
                            else:
                                nc.scalar.activation(out=rsig[:, co, :], in_=ps,
                                                     func=AF.Sigmoid)
                    dma8_out(ekd[ic], ek, 4)
                    dma8_out(ekvd[ic], ekv, 4)
                    dma8_out(rsd1[ic], rsig, 2)

              # ================= Phase 1b: WKV scan + Wo + residual ============
              with contextlib.ExitStack() as p1b:
                wpool = p1b.enter_context(tc.tile_pool(name="w1b", bufs=1))
                act = p1b.enter_context(tc.tile_pool(name="act1b", bufs=1))
                dbl = p1b.enter_context(tc.tile_pool(name="dbl1b", bufs=2))
                ps_ev = p1b.enter_context(tc.tile_pool(name="ps_evb", bufs=4, space="PSUM"))

                wo_t = wpool.tile([128, CB, C], bf16, tag="wo")
                dma8(wo_t, Wo.rearrange("(a p) m -> p a m", p=128))

                for ic in range(NCH):
                    t0 = ic * TC
                    ek = dbl.tile([128, CB, TC], f32, tag="ekb")
                    dma8(ek, ekd[ic], parts=4)
                    ekv = dbl.tile([128, CB, TC], f32, tag="ekvb")
                    dma8(ekv, ekvd[ic], parts=4)
                    rsig = dbl.tile([128, CB, TC], bf16, tag="rsigb")
                    dma8(rsig, rsd1[ic], parts=2)
                    x_t = dbl.tile([128, CB, TC], f32, tag="xb")
                    dma8(x_t, xT.bitcast(f32).rearrange("(cb p) t -> p cb t", p=128)[:, :, t0:t0 + TC], parts=4)

                    A_t = dbl.tile([128, CB, TC + 1], f32, tag="A")
                    B_t = dbl.tile([128, CB, TC + 1], f32, tag="B")
                    nc.vector.tensor_copy(out=A_t[:, :, 0:1], in_=carryA)
                    nc.vector.tensor_copy(out=B_t[:, :, 0:1], in_=carryB)
                    for cb in range(CB):
                        ew_b = _bcast_free(cv[:, cb, EW:EW + 1], TC)
                        nc.vector.tensor_tensor_scan(
                            out=A_t[:, cb, 1:TC + 1], data0=ew_b, data1=ekv[:, cb, :],
                            initial=A_t[:, cb, 0:1], op0=OP.mult, op1=OP.add)
                        nc.vector.tensor_tensor_scan(
                            out=B_t[:, cb, 1:TC + 1], data0=ew_b, data1=ek[:, cb, :],
                            initial=B_t[:, cb, 0:1], op0=OP.mult, op1=OP.add)
                    nc.vector.tensor_copy(out=carryA, in_=A_t[:, :, TC:TC + 1])
                    nc.vector.tensor_copy(out=carryB, in_=B_t[:, :, TC:TC + 1])

                    # num -> ekv, den -> ek (in place)
                    for cb in range(CB):
                        eu_s = cv[:, cb, EU:EU + 1]
                        nc.vector.scalar_tensor_tensor(
                            out=ekv[:, cb, :], in0=ekv[:, cb, :], scalar=eu_s,
                            in1=A_t[:, cb, 0:TC], op0=OP.mult, op1=OP.add)
                        nc.vector.scalar_tensor_tensor(
                            out=ek[:, cb, :], in0=ek[:, cb, :], scalar=eu_s,
                            in1=B_t[:, cb, 0:TC], op0=OP.mult, op1=OP.add)
                    nc.vector.reciprocal_approx_fast(out=ek, in_=ek)
                    nc.gpsimd.tensor_mul(ekv, ekv, ek)          # wkv
                    y_t = dbl.tile([128, CB, TC], bf16, tag="yb")
                    nc.vector.tensor_mul(y_t, ekv, rsig)        # r_sig * wkv

                    x2_t = dbl.tile([128, CB, TC], f32, tag="x2a")
                    for co in range(CB):
                        ps = ps_ev.tile([128, TC], f32, tag="ev")
                        csl = slice(co * 128, (co + 1) * 128)
                        for a in range(CB):
                            nc.tensor.matmul(ps, wo_t[:, a, csl], y_t[:, a, :],
                                             start=(a == 0), stop=(a == CB - 1))
                        nc.vector.tensor_add(x2_t[:, co, :], x_t[:, co, :], ps)
                    dma8_out(x2d[ic], x2_t, 4)

            # ================= Phase 2a: FFN kk/rr production =================
            if "2" in _PH:
              kkd = dramp.tile([NCH, 128, FB, TC], bf16, tag="kkd")
              rsd = dramp.tile([NCH, 128, CB, TC], bf16, tag="rsd")
              with contextlib.ExitStack() as p2:
                wpool = p2.enter_context(tc.tile_pool(name="w2", bufs=1))
                act = p2.enter_context(tc.tile_pool(name="act2", bufs=1))
                dbl = p2.enter_context(tc.tile_pool(name="dbl2", bufs=2))
                scratch = p2.enter_context(tc.tile_pool(name="scr2", bufs=1))
                rel = p2.enter_context(tc.tile_pool(name="rel", bufs=2))
                ps_ev = p2.enter_context(tc.tile_pool(name="ps_ev2", bufs=4, space="PSUM"))
                ps_stat = p2.enter_context(tc.tile_pool(name="ps_st2", bufs=1, space="PSUM"))
                ps_bc = p2.enter_context(tc.tile_pool(name="ps_bc2", bufs=1, space="PSUM"))

                fwk_t = wpool.tile([128, CB, 4 * C], bf16, tag="fwk")
                dma8(fwk_t, fWk.rearrange("(a p) m -> p a m", p=128))
                fwr_t = wpool.tile([128, CB, C], bf16, tag="fwr")
                dma8(fwr_t, fWr.rearrange("(a p) m -> p a m", p=128))

                for ic in range(NCH):
                    x_t = dbl.tile([128, CB, TC], f32r, tag="x2")
                    dma8(x_t, x2d[ic].bitcast(f32r), parts=4)
                    g_t = dbl.tile([128, CB, TC + 1], f32, tag="g")
                    nc.vector.tensor_copy(out=g_t[:, :, 0:1], in_=carryG)
                    layernorm((dbl, ps_stat, ps_bc, scratch, "d2", "d2"), x_t,
                              lambda cb: cv[:, cb, LN2G:LN2G + 1],
                              lambda cb: cv[:, cb, LN2B:LN2B + 1], g_t)
                    nc.vector.tensor_copy(out=carryG, in_=g_t[:, :, TC:TC + 1])

                    d_t = dbl.tile([128, CB, TC], f32, tag="d2")
                    nc.vector.tensor_sub(d_t, g_t[:, :, 1:TC + 1], g_t[:, :, 0:TC])
                    fin_k = dbl.tile([128, CB, TC], bf16, tag="fink")
                    fin_r = dbl.tile([128, CB, TC], bf16, tag="finr")
                    for cb in range(CB):
                        nc.vector.scalar_tensor_tensor(
                            out=fin_k[:, cb, :], in0=d_t[:, cb, :],
                            scalar=cv[:, cb, FTMK:FTMK + 1],
                            in1=g_t[:, cb, 0:TC], op0=OP.mult, op1=OP.add)
                        nc.vector.scalar_tensor_tensor(
                            out=fin_r[:, cb, :], in0=d_t[:, cb, :],
                            scalar=cv[:, cb, FTMR:FTMR + 1],
                            in1=g_t[:, cb, 0:TC], op0=OP.mult, op1=OP.add)

                    kk = dbl.tile([128, FB, TC], bf16, tag="kk")
                    for co in range(FB):
                        ps = ps_ev.tile([128, TC], f32, tag="ev2")
                        csl = slice(co * 128, (co + 1) * 128)
                        for a in range(CB):
                            nc.tensor.matmul(ps, fwk_t[:, a, csl], fin_k[:, a, :],
                                             start=(a == 0), stop=(a == CB - 1))
                        rt = rel.tile([128, TC], f32, tag="rt")
                        nc.scalar.activation(out=rt, in_=ps, func=AF.Relu)
                        nc.vector.tensor_mul(kk[:, co, :], rt, rt)
                    dma8_out(kkd[ic], kk, 8)

                    rsig2 = dbl.tile([128, CB, TC], bf16, tag="rsig2")
                    for co in range(CB):
                        ps = ps_ev.tile([128, TC], f32, tag="ev2")
                        csl = slice(co * 128, (co + 1) * 128)
                        for a in range(CB):
                            nc.tensor.matmul(ps, fwr_t[:, a, csl], fin_r[:, a, :],
                                             start=(a == 0), stop=(a == CB - 1))
                        nc.scalar.activation(out=rsig2[:, co, :], in_=ps, func=AF.Sigmoid)
                    dma8_out(rsd[ic], rsig2, 2)

              # ================= Phase 2b: FFN down-proj + residual ============
              with contextlib.ExitStack() as p2b:
                wpool = p2b.enter_context(tc.tile_pool(name="w2b", bufs=1))
                dbl = p2b.enter_context(tc.tile_pool(name="dbl2b", bufs=2))
                ps_ffn = p2b.enter_context(tc.tile_pool(name="ps_ffn", bufs=4, space="PSUM"))

                fwv_t = wpool.tile([128, FB, C], bf16, tag="fwv")
                dma8(fwv_t, fWv.rearrange("(a p) m -> p a m", p=128), parts=16)

                for ic in range(NCH):
                    t0 = ic * TC
                    kk = dbl.tile([128, FB, TC], bf16, tag="kkb")
                    dma8(kk, kkd[ic], parts=8)
                    rsig2 = dbl.tile([128, CB, TC], bf16, tag="rsig2b")
                    dma8(rsig2, rsd[ic], parts=2)
                    x2_t = dbl.tile([128, CB, TC], f32, tag="x2b")
                    dma8(x2_t, x2d[ic], parts=4)
                    out_t = dbl.tile([128, CB, TC], f32, tag="outb")
                    for co in range(CB):
                        ps = ps_ffn.tile([128, TC], f32, tag="ffn")
                        csl = slice(co * 128, (co + 1) * 128)
                        for a in range(FB):
                            nc.tensor.matmul(ps, fwv_t[:, a, csl], kk[:, a, :],
                                             start=(a == 0), stop=(a == FB - 1))
                        nc.vector.tensor_mul(ps, rsig2[:, co, :], ps)
                        nc.vector.tensor_add(out_t[:, co, :], x2_t[:, co, :], ps)
                    dma8_out(outT.rearrange("(cb p) t -> p cb t", p=128)[:, :, t0:t0 + TC], out_t, 4)

    nc.finalize()
    return nc


def _prep_maps(inputs):
    x = np.asarray(inputs["x"], np.float32)
    ew = np.exp(-np.exp(np.asarray(inputs["time_decay"], np.float32))).astype(np.float32)
    eu = np.exp(np.asarray(inputs["time_first"], np.float32)).astype(np.float32)
    cvecs = np.stack([
        np.asarray(inputs["ln1_g"], np.float32), np.asarray(inputs["ln1_b"], np.float32),
        np.asarray(inputs["ln2_g"], np.float32), np.asarray(inputs["ln2_b"], np.float32),
        np.asarray(inputs["tmk"], np.float32), np.asarray(inputs["tmv"], np.float32),
        np.asarray(inputs["tmr"], np.float32), np.asarray(inputs["ftmk"], np.float32),
        np.asarray(inputs["ftmr"], np.float32), ew, eu,
        np.zeros(C, np.float32),
    ]).astype(np.float32)
    cvecs = np.ascontiguousarray(cvecs.reshape(12, CB, 128).transpose(2, 1, 0))
    common = {
        "cvecs": cvecs,
        "ones128": np.ones(128, np.float32),
        "Wk": np.asarray(inputs["Wk"], np.float32),
        "Wv": np.asarray(inputs["Wv"], np.float32),
        "Wr": np.asarray(inputs["Wr"], np.float32),
        "Wo": np.asarray(inputs["Wo"]).astype(ml_dtypes.bfloat16),
        "fWk": np.asarray(inputs["fWk"]).astype(ml_dtypes.bfloat16),
        "fWv": np.asarray(inputs["fWv"]).astype(ml_dtypes.bfloat16),
        "fWr": np.asarray(inputs["fWr"]).astype(ml_dtypes.bfloat16),
    }
    return [{**common, "xT": np.ascontiguousarray(x[b].T)} for b in range(B)]


def get_nc():
    if "nc" not in _CACHE:
        _CACHE["nc"] = _build()
    return _CACHE["nc"]


def kernel(**inputs):
    from concourse.bass_utils import run_bass_kernel_spmd
    nc = get_nc()
    in_maps = _prep_maps(inputs)
    res = run_bass_kernel_spmd(nc, in_maps, core_ids=list(range(B)))
    return np.stack([np.ascontiguousarray(r["outT"].T) for r in res.results])

